# revision 1
# baseline (speedup 1.0000x reference)
"""ConvLSTM decoder Trainium2 kernel.

Strategy
--------
Data-parallel over batch: 64 images -> 8 NeuronCores x 8 images. Conv/dense
weights replicated on every core. The T=48 recurrence runs as a `For_i` loop
on-device; each step does two ConvLSTM layers.

Conv3x3(SAME) is computed as 9 shifted matmuls accumulating in PSUM:
  z[gate*128+m, img, y, x] = sum_{dy,dx,c} W[.., c, dy, dx] * in[c, img, y+dy-1, x+dx-1]
with the input planes stored zero-padded (17x17) in SBUF, channels on
partitions. Contraction over input channels (<=128 per chunk) maps to the PE
partition dim; each gate is exactly one 128-partition output chunk. Matmul
free dim = 2 images x 225 pixels = 450 (fits one PSUM bank).

All matmul operands are bf16 (fp32 PSUM accumulation); gate math + cell state
are fp32. Measured numerics vs the fp32 reference: ~0.35% L2 rel err.

Host-side prep (inside kernel()): shard the batch, zero-pad + cast x to bf16,
repack weights into lhsT layouts. Device does all conv/LSTM/dense compute.

NOTE on DMA count: walrus caps sync-wait commands per instruction; the For_i
back-edge drain waits on one sem per DMAHW lane ever used (round-robin over
8) plus one per engine. Keep the total number of dma_start calls <= 4 so only
lanes 0-3 exist: all weights ride in one DMA, all biases in another, x is one
DMA per step (same static instruction = same lane), output one.
"""

import numpy as np
import ml_dtypes

import concourse.bass as bass
from concourse import bacc
import concourse.mybir as mybir
import concourse.tile as tile
from concourse.bass import ds
from concourse.bass_utils import run_bass_kernel_spmd

BF16 = ml_dtypes.bfloat16
F32 = mybir.dt.float32
BF = mybir.dt.bfloat16

# Problem constants (hardcoded per contract).
B, T, C_IN, H, W = 64, 48, 64, 15, 15
HID, KK, OUT = 128, 3, 128
NCORES = 8
BC = B // NCORES          # images per core = 8
PH, PW = H + 2, W + 2     # padded plane 17x17
PP = PH * PW              # 289
ROW = BC * PP             # 2312 free elems in padded activations
S = H * W                 # 225
NPAIR = BC // 2           # 4 image pairs
NF = 2 * S                # 450 matmul free dim
WCONV = 9 * 2 * 512       # per-layer conv weight free size (9216)
WALL = 2 * WCONV + S * OUT  # 47232
AFT = mybir.ActivationFunctionType


def build_nc(t_steps: int = T) -> bass.Bass:
    nc = bacc.Bacc("TRN2", target_bir_lowering=False, debug=False)

    x_d = nc.dram_tensor("x", [t_steps * C_IN, ROW], BF, kind="ExternalInput")
    w_d = nc.dram_tensor("w", [128, WALL], BF, kind="ExternalInput")
    b_d = nc.dram_tensor("b", [128, 9], F32, kind="ExternalInput")
    out_d = nc.dram_tensor("out", [OUT, BC], F32, kind="ExternalOutput")

    # Persistent SBUF state.
    xsb = nc.alloc_sbuf_tensor("xsb", [128, ROW], BF)     # x_t padded (rows 64+ zero)
    h1p = nc.alloc_sbuf_tensor("h1p", [128, ROW], BF)     # layer-1 hidden, padded
    h2p = nc.alloc_sbuf_tensor("h2p", [128, ROW], BF)     # layer-2 hidden, padded
    c1 = nc.alloc_sbuf_tensor("c1", [128, BC * S], F32)
    c2 = nc.alloc_sbuf_tensor("c2", [128, BC * S], F32)
    wsb = nc.alloc_sbuf_tensor("wsb", [128, WALL], BF)    # w0 | w1 | wd
    bsb = nc.alloc_sbuf_tensor("bsb", [128, 9], F32)      # b0 | b1 | bd

    def padded(ap):
        return ap.rearrange("p (i y x) -> p i y x", i=BC, y=PH, x=PW)

    w0v = wsb.ap()[:, 0:WCONV].rearrange("p (t k o) -> p t k o", t=9, k=2, o=512)
    w1v = wsb.ap()[:, WCONV:2 * WCONV].rearrange("p (t k o) -> p t k o", t=9, k=2, o=512)
    wdv = wsb.ap()[:, 2 * WCONV:].rearrange("p (s o) -> p s o", s=S, o=OUT)

    # The walrus codegen caps sync-wait commands per instruction (~4-5); a
    # For_i back-edge drain waits on one sem per engine + one per DMA lane
    # used anywhere in its TileContext. Split the program into three
    # sequential TileContexts so each context's drains see few sems: the
    # loop context contains exactly one DMA instruction (the x load).
    with tile.TileContext(nc) as tc:
        # --- preamble: zero state + load weights ---
        nc.vector.memset(xsb.ap()[64:128, :], 0.0)
        nc.vector.memset(h1p.ap()[:, :], 0.0)
        nc.vector.memset(h2p.ap()[:, :], 0.0)
        nc.vector.memset(c1.ap()[:, :], 0.0)
        nc.vector.memset(c2.ap()[:, :], 0.0)
        nc.sync.dma_start(wsb.ap()[:, :], w_d.ap()[:, :])
        nc.sync.dma_start(bsb.ap()[:, :], b_d.ap()[:, :])

    with tile.TileContext(nc) as tc:
        with (
            tc.tile_pool(name="psum", bufs=8, space="PSUM") as psum,
            tc.tile_pool(name="gates", bufs=5) as gates,
            tc.tile_pool(name="tmps", bufs=3) as tmps,
        ):
            def lstm_layer(inp, selfp, cst, wv, bofs):
                """One ConvLSTM step. inp = input planes (kc=0 weights),
                selfp = this layer's hidden planes (kc=1); writes selfp."""
                inv = padded(inp.ap())
                selfv = padded(selfp.ap())
                gsb = []
                for g in range(4):
                    pts = [psum.tile([128, NF], F32, tag="ps", name=f"ps{g}_{i}")
                           for i in range(NPAIR)]
                    for kc, src in ((1, selfv), (0, inv)):
                        for tap in range(9):
                            dy, dx = divmod(tap, 3)
                            lhsT = wv[:, tap, kc, g * 128:(g + 1) * 128]
                            for ip in range(NPAIR):
                                rhs = src[:, 2 * ip:2 * ip + 2,
                                          dy:dy + H, dx:dx + W]
                                nc.tensor.matmul(
                                    pts[ip][:, :], lhsT, rhs,
                                    start=(kc == 1 and tap == 0),
                                    stop=(kc == 0 and tap == 8),
                                )
                    gt = gates.tile([128, BC * S], F32, tag="gate", name=f"g{g}")
                    func = AFT.Tanh if g == 3 else AFT.Sigmoid
                    for ip in range(NPAIR):
                        nc.scalar.activation(
                            gt[:, ip * NF:(ip + 1) * NF], pts[ip][:, :],
                            func, bias=bsb.ap()[:, bofs + g:bofs + g + 1])
                    gsb.append(gt)
                gi, gf, go, gg = gsb
                t1 = tmps.tile([128, BC * S], F32, tag="tmp", name="t1")
                t2 = tmps.tile([128, BC * S], F32, tag="tmp", name="t2")
                tch = tmps.tile([128, BC * S], F32, tag="tmp", name="tch")
                nc.vector.tensor_mul(t1[:, :], gf[:, :], cst.ap()[:, :])
                nc.vector.tensor_mul(t2[:, :], gi[:, :], gg[:, :])
                nc.vector.tensor_add(cst.ap()[:, :], t1[:, :], t2[:, :])
                nc.scalar.activation(tch[:, :], cst.ap()[:, :], AFT.Tanh)
                hdst = selfv[:, :, 1:1 + H, 1:1 + W]
                ov = go[:, :].rearrange("p (i y x) -> p i y x", i=BC, y=H, x=W)
                tv = tch[:, :].rearrange("p (i y x) -> p i y x", i=BC, y=H, x=W)
                nc.vector.tensor_mul(hdst, ov, tv)

            x2 = x_d.ap()
            with tc.For_i(0, t_steps * C_IN, C_IN) as iv:
                nc.sync.dma_start(xsb.ap()[0:C_IN, :], x2[ds(iv, C_IN), :])
                lstm_layer(xsb, h1p, c1, w0v, 0)
                lstm_layer(h1p, h2p, c2, w1v, 4)

    with tile.TileContext(nc) as tc:
        with (
            tc.tile_pool(name="psum2", bufs=1, space="PSUM") as psum2,
            tc.tile_pool(name="outp", bufs=1) as outp,
        ):
            # Dense head: out[o, img] = sum_{c,s} h2[c, img, s] * Wd[c*225+s, o]
            h2v = padded(h2p.ap())
            po = psum2.tile([128, BC], F32, tag="ps", name="po")
            for s in range(S):
                py, px = divmod(s, 15)
                rhs = h2v[:, :, 1 + py, 1 + px]
                nc.tensor.matmul(po[:, :], wdv[:, s, :], rhs,
                                 start=(s == 0), stop=(s == S - 1))
            osb = outp.tile([128, BC], F32, tag="o", name="osb")
            nc.scalar.activation(osb[:, :], po[:, :], AFT.Identity,
                                 bias=bsb.ap()[:, 8:9])
            nc.sync.dma_start(out_d.ap()[:, :], osb[:, :])

    nc.compile()
    return nc


def pack_inputs(inputs: dict, t_steps: int = T) -> tuple[list[dict], dict]:
    """Host-side layout prep. Returns (per_core_in_maps, shared_tensors)."""
    enc = np.ascontiguousarray(np.asarray(inputs["encoder_output"], np.float32))
    W0 = np.asarray(inputs["W0"], np.float32)
    W1 = np.asarray(inputs["W1"], np.float32)
    b0 = np.asarray(inputs["b0"], np.float32)
    b1 = np.asarray(inputs["b1"], np.float32)
    Wd = np.asarray(inputs["Wd"], np.float32)
    bd = np.asarray(inputs["bd"], np.float32)

    def pack_conv(Wc, cin0):
        # Wc: [512, cin0+128, 3, 3] -> [128, 9*2*512] (k, (tap, kchunk, o))
        Wr = Wc.reshape(512, Wc.shape[1], 9)
        w = np.zeros((128, 9, 2, 512), np.float32)
        w[:cin0, :, 0, :] = Wr[:, :cin0].transpose(1, 2, 0)
        w[:, :, 1, :] = Wr[:, cin0:cin0 + 128].transpose(1, 2, 0)
        return w.reshape(128, WCONV)

    wall = np.concatenate(
        [pack_conv(W0, C_IN), pack_conv(W1, HID), Wd.reshape(HID, S * OUT)],
        axis=1).astype(BF16)
    ball = np.concatenate(
        [b0.reshape(4, 128).T, b1.reshape(4, 128).T, bd.reshape(128, 1)],
        axis=1).astype(np.float32)
    ball = np.ascontiguousarray(ball)

    shared = {"w": wall, "b": ball}
    in_maps = []
    for c in range(NCORES):
        xc = enc[c * BC:(c + 1) * BC, :t_steps]          # [8, t, 64, 15, 15]
        xp = np.zeros((t_steps, C_IN, BC, PH, PW), BF16)
        xp[:, :, :, 1:1 + H, 1:1 + W] = xc.transpose(1, 2, 0, 3, 4)
        in_maps.append({"x": xp.reshape(t_steps * C_IN, ROW), **shared})
    return in_maps, shared


def kernel(**inputs) -> np.ndarray:
    nc = build_nc(T)
    in_maps, _ = pack_inputs(inputs, T)
    res = run_bass_kernel_spmd(nc, in_maps, list(range(NCORES))).results
    out = np.concatenate([np.asarray(r["out"], np.float32).T for r in res], axis=0)
    return np.ascontiguousarray(out)


if __name__ == "__main__":
    ins = {k: np.asarray(v) for k, v in np.load("inputs.npz").items()}
    out = kernel(**ins)
    exp = np.load("expected.npy")
    d = out - exp
    print("rel l2:", np.linalg.norm(d) / np.linalg.norm(exp))



# revision 2
# speedup vs baseline: 1.0117x; 1.0117x over previous
"""ConvLSTM decoder Trainium2 kernel, v2.

Strategy (changes vs v1)
------------------------
- Pair-major scheduling: per 2-image pair, run all 4 gates' matmuls
  back-to-back, then that pair's activations + cell math overlap the next
  pair's matmuls.  The PE never waits on gate math at layer boundaries.
- Layer-1 input projection uses host-side im2col: x (64 ch, not recurrent)
  is repacked on the host to (tap, channel) rows -> 4.5 dense 128-row
  contraction chunks instead of 9 half-empty taps.  Per step: L1 = 4 gates
  x (5 x-chunks + 9 h-taps), L2 = 4 gates x 18 taps -> 512 matmuls/step
  (was 576).
- fp16 operands instead of bf16 (same PE throughput, 3 extra mantissa bits).

Layout
------
x_cols DRAM [T*128, 5*1800] fp16: per step a [128, 5, 1800] SBUF tile;
rows = (tap*64+c) mod 128 within chunk k = (tap*64+c)//128; chunk 4 has
rows 64..127 zero.  Conv weights for h-taps / layer-2 as in v1; layer-1
x weights packed to the matching [128, 5, 512] chunk layout.

DMA count stays at 4 static instructions (w, b, x_cols in-loop, out) to
respect the walrus sync-wait cap on the For_i back-edge drain.
"""

import numpy as np

import concourse.bass as bass
from concourse import bacc
import concourse.mybir as mybir
import concourse.tile as tile
from concourse.bass import ds
from concourse.bass_utils import run_bass_kernel_spmd

F32 = mybir.dt.float32
F16 = mybir.dt.float16
NPF16 = np.float16

B, T, C_IN, H, W = 64, 48, 64, 15, 15
HID, KK, OUT = 128, 3, 128
NCORES = 8
BC = B // NCORES          # images per core = 8
PH, PW = H + 2, W + 2     # padded plane 17x17
PP = PH * PW              # 289
ROW = BC * PP             # 2312
S = H * W                 # 225
NPAIR = BC // 2           # 4
NF = 2 * S                # 450 matmul free dim
NXCH = 5                  # x im2col chunks (576 rows -> 5x128, last half-empty)
W0H = 9 * 512             # layer-1 h-tap weights, per-partition elems
W0X = NXCH * 512          # layer-1 x im2col weights
W1C = 9 * 2 * 512         # layer-2 conv weights
WALL = W0H + W0X + W1C + S * OUT
AFT = mybir.ActivationFunctionType


def build_nc(t_steps: int = T) -> bass.Bass:
    nc = bacc.Bacc("TRN2", target_bir_lowering=False, debug=False)

    x_d = nc.dram_tensor("x", [t_steps * 128, NXCH * 2 * S * NPAIR], F16,
                         kind="ExternalInput")
    w_d = nc.dram_tensor("w", [128, WALL], F16, kind="ExternalInput")
    b_d = nc.dram_tensor("b", [128, 9], F32, kind="ExternalInput")
    out_d = nc.dram_tensor("out", [OUT, BC], F32, kind="ExternalOutput")

    # Persistent SBUF state.
    h1p = nc.alloc_sbuf_tensor("h1p", [128, ROW], F16)    # layer-1 hidden, padded
    h2p = nc.alloc_sbuf_tensor("h2p", [128, ROW], F16)    # layer-2 hidden, padded
    c1 = nc.alloc_sbuf_tensor("c1", [128, BC * S], F32)
    c2 = nc.alloc_sbuf_tensor("c2", [128, BC * S], F32)
    wsb = nc.alloc_sbuf_tensor("wsb", [128, WALL], F16)   # w0h | w0x | w1 | wd
    bsb = nc.alloc_sbuf_tensor("bsb", [128, 9], F32)      # b0 | b1 | bd

    def padded(ap):
        return ap.rearrange("p (i y x) -> p i y x", i=BC, y=PH, x=PW)

    w0hv = wsb.ap()[:, 0:W0H].rearrange("p (t o) -> p t o", t=9, o=512)
    w0xv = wsb.ap()[:, W0H:W0H + W0X].rearrange("p (k o) -> p k o", k=NXCH, o=512)
    w1v = wsb.ap()[:, W0H + W0X:W0H + W0X + W1C].rearrange(
        "p (t k o) -> p t k o", t=9, k=2, o=512)
    wdv = wsb.ap()[:, W0H + W0X + W1C:].rearrange("p (s o) -> p s o", s=S, o=OUT)

    with tile.TileContext(nc) as tc:
        nc.vector.memset(h1p.ap()[:, :], 0.0)
        nc.vector.memset(h2p.ap()[:, :], 0.0)
        nc.vector.memset(c1.ap()[:, :], 0.0)
        nc.vector.memset(c2.ap()[:, :], 0.0)
        nc.sync.dma_start(wsb.ap()[:, :], w_d.ap()[:, :])
        nc.sync.dma_start(bsb.ap()[:, :], b_d.ap()[:, :])

    with tile.TileContext(nc) as tc:
        with (
            tc.tile_pool(name="psum", bufs=8, space="PSUM") as psum,
            tc.tile_pool(name="xin", bufs=2) as xin,
            tc.tile_pool(name="gates", bufs=12) as gates,
            tc.tile_pool(name="tmps", bufs=6) as tmps,
        ):
            def pair_block(layer, ip, xcols):
                """All 4 gates for one image pair of one layer + cell math."""
                if layer == 0:
                    inv, selfv, cst, bofs = None, padded(h1p.ap()), c1, 0
                else:
                    inv, selfv, cst, bofs = padded(h1p.ap()), padded(h2p.ap()), c2, 4
                sl = slice(ip * NF, (ip + 1) * NF)
                gsb = []
                for g in range(4):
                    ps = psum.tile([128, NF], F32, tag="ps", name=f"ps{layer}{ip}{g}")
                    n = 0
                    nmm = 18 if layer else 14
                    # self-hidden taps first (ready since last step)
                    for tap in range(9):
                        dy, dx = divmod(tap, 3)
                        lhsT = (w1v[:, tap, 1, g * 128:(g + 1) * 128] if layer
                                else w0hv[:, tap, g * 128:(g + 1) * 128])
                        rhs = selfv[:, 2 * ip:2 * ip + 2, dy:dy + H, dx:dx + W]
                        nc.tensor.matmul(ps[:, :], lhsT, rhs,
                                         start=(n == 0), stop=(n == nmm - 1))
                        n += 1
                    if layer == 0:
                        for k in range(NXCH):
                            nc.tensor.matmul(
                                ps[:, :], w0xv[:, k, g * 128:(g + 1) * 128],
                                xcols[:, k, sl],
                                start=False, stop=(n == nmm - 1))
                            n += 1
                    else:
                        for tap in range(9):
                            dy, dx = divmod(tap, 3)
                            rhs = inv[:, 2 * ip:2 * ip + 2, dy:dy + H, dx:dx + W]
                            nc.tensor.matmul(
                                ps[:, :], w1v[:, tap, 0, g * 128:(g + 1) * 128],
                                rhs, start=False, stop=(n == nmm - 1))
                            n += 1
                    gt = gates.tile([128, NF], F32, tag="gate",
                                    name=f"g{layer}{ip}{g}")
                    func = AFT.Tanh if g == 3 else AFT.Sigmoid
                    nc.scalar.activation(gt[:, :], ps[:, :], func,
                                         bias=bsb.ap()[:, bofs + g:bofs + g + 1])
                    gsb.append(gt)
                gi, gf, go, gg = gsb
                t1 = tmps.tile([128, NF], F32, tag="tmp", name="t1")
                t2 = tmps.tile([128, NF], F32, tag="tmp", name="t2")
                tch = tmps.tile([128, NF], F32, tag="tmp", name="tch")
                cs = cst.ap()[:, ip * NF:(ip + 1) * NF]
                nc.vector.tensor_mul(t1[:, :], gf[:, :], cs)
                nc.vector.tensor_mul(t2[:, :], gi[:, :], gg[:, :])
                nc.vector.tensor_add(cs, t1[:, :], t2[:, :])
                nc.scalar.activation(tch[:, :], cs, AFT.Tanh)
                hdst = selfv[:, 2 * ip:2 * ip + 2, 1:1 + H, 1:1 + W]
                ov = go[:, :].rearrange("p (i y x) -> p i y x", i=2, y=H, x=W)
                tv = tch[:, :].rearrange("p (i y x) -> p i y x", i=2, y=H, x=W)
                nc.vector.tensor_mul(hdst, ov, tv)

            x2 = x_d.ap()
            with tc.For_i(0, t_steps * 128, 128) as iv:
                xt = xin.tile([128, NXCH, NF * NPAIR], F16, tag="x", name="xt")
                nc.sync.dma_start(
                    xt[:, :, :],
                    x2[ds(iv, 128), :].rearrange("p (k f) -> p k f", k=NXCH))
                for layer in range(2):
                    for ip in range(NPAIR):
                        pair_block(layer, ip, xt)

    with tile.TileContext(nc) as tc:
        with (
            tc.tile_pool(name="psum2", bufs=1, space="PSUM") as psum2,
            tc.tile_pool(name="outp", bufs=1) as outp,
        ):
            h2v = padded(h2p.ap())
            po = psum2.tile([128, BC], F32, tag="ps", name="po")
            for s in range(S):
                py, px = divmod(s, 15)
                rhs = h2v[:, :, 1 + py, 1 + px]
                nc.tensor.matmul(po[:, :], wdv[:, s, :], rhs,
                                 start=(s == 0), stop=(s == S - 1))
            osb = outp.tile([128, BC], F32, tag="o", name="osb")
            nc.scalar.activation(osb[:, :], po[:, :], AFT.Identity,
                                 bias=bsb.ap()[:, 8:9])
            nc.sync.dma_start(out_d.ap()[:, :], osb[:, :])

    nc.compile()
    return nc


def pack_inputs(inputs: dict, t_steps: int = T) -> tuple[list[dict], dict]:
    """Host-side layout prep. Returns (per_core_in_maps, shared_tensors)."""
    enc = np.ascontiguousarray(np.asarray(inputs["encoder_output"], np.float32))
    W0 = np.asarray(inputs["W0"], np.float32)
    W1 = np.asarray(inputs["W1"], np.float32)
    b0 = np.asarray(inputs["b0"], np.float32)
    b1 = np.asarray(inputs["b1"], np.float32)
    Wd = np.asarray(inputs["Wd"], np.float32)
    bd = np.asarray(inputs["bd"], np.float32)

    # Layer-1 h-tap weights: W0[:, 64:192] -> [128, 9, 512]
    w0h = W0[:, C_IN:].reshape(512, 128, 9).transpose(1, 2, 0)
    # Layer-1 x weights, im2col rows r = tap*64 + c -> [640 -> 5*128, 512]
    w0x = np.zeros((NXCH * 128, 512), np.float32)
    w0x[:9 * C_IN] = W0[:, :C_IN].reshape(512, C_IN, 9).transpose(2, 1, 0).reshape(9 * C_IN, 512)
    w0x = w0x.reshape(NXCH, 128, 512).transpose(1, 0, 2).reshape(128, W0X)
    # Layer-2 weights: [128, 9, 2, 512] (k=0 input h1, k=1 self h2)
    W1r = W1.reshape(512, 256, 9)
    w1 = np.empty((128, 9, 2, 512), np.float32)
    w1[:, :, 0, :] = W1r[:, :128].transpose(1, 2, 0)
    w1[:, :, 1, :] = W1r[:, 128:].transpose(1, 2, 0)

    wall = np.concatenate(
        [w0h.reshape(128, W0H), w0x, w1.reshape(128, W1C),
         Wd.reshape(HID, S * OUT)], axis=1).astype(NPF16)
    ball = np.concatenate(
        [b0.reshape(4, 128).T, b1.reshape(4, 128).T, bd.reshape(128, 1)],
        axis=1).astype(np.float32)
    ball = np.ascontiguousarray(ball)

    shared = {"w": wall, "b": ball}
    in_maps = []
    for c in range(NCORES):
        xc = enc[c * BC:(c + 1) * BC, :t_steps]            # [8, t, 64, 15, 15]
        xp = np.zeros((t_steps, C_IN, BC, PH, PW), np.float32)
        xp[:, :, :, 1:1 + H, 1:1 + W] = xc.transpose(1, 2, 0, 3, 4)
        # im2col: rows (tap, c), cols (img, y, x)
        cols = np.zeros((t_steps, NXCH * 128, BC * S), NPF16)
        for tap in range(9):
            dy, dx = divmod(tap, 3)
            sh = xp[:, :, :, dy:dy + H, dx:dx + W].reshape(
                t_steps, C_IN, BC * S)
            cols[:, tap * C_IN:(tap + 1) * C_IN] = sh
        xcols = cols.reshape(t_steps, NXCH, 128, BC * S).transpose(
            0, 2, 1, 3).reshape(t_steps * 128, NXCH * BC * S)
        in_maps.append({"x": np.ascontiguousarray(xcols), **shared})
    return in_maps, shared


def kernel(**inputs) -> np.ndarray:
    nc = build_nc(T)
    in_maps, _ = pack_inputs(inputs, T)
    res = run_bass_kernel_spmd(nc, in_maps, list(range(NCORES))).results
    out = np.concatenate([np.asarray(r["out"], np.float32).T for r in res], axis=0)
    return np.ascontiguousarray(out)


if __name__ == "__main__":
    ins = {k: np.asarray(v) for k, v in np.load("inputs.npz").items()}
    out = kernel(**ins)
    exp = np.load("expected.npy")
    d = out - exp
    print("rel l2:", np.linalg.norm(d) / np.linalg.norm(exp))
